# revision 32
# baseline (speedup 1.0000x reference)
"""Multi-head attention (B=2, T=2048, H=1024, 16 heads) on 8 trn2 cores.

Sharding: data-parallel over batch (2) x tensor-parallel over head groups
(4 heads/core).  Each core computes qkv projection for its 4 heads,
attention, and a partial out-projection; the host sums 4 partials per
batch and adds b_out.

Key optimizations over the f32r v1 kernel:
- The mask zeroes entire key tokens for every query, so the host compacts
  K/V inputs to the ~1024 unmasked tokens, padded to TK=1152 (9 tiles).
  Scores, exp and AV shrink by 7/16.  Pad slots get maskbias=-1e9.
- All matmul operands are bf16 (PE rate is the same as f32r but FWL
  halves weight loads, and SBUF/DMA traffic halves).  PSUM stays f32.
- x is transposed on the host; no on-device transposes at all.  V is
  projected directly in natural [token, feat] orientation.
- Scores for the two heads of a pair run as row-tiled K=64 matmuls
  (PE partitions 0-63 / 64-127) writing one [128,1024] PSUM tile; a
  single exp activation (mask bias per partition) covers both heads.
- V carries a ones column so the AV matmul accumulates the softmax
  denominator in row 64 of the PSUM acc; denominator rows are copied to
  SBUF (DVE), batched into one reciprocal per pair, PE-broadcast, and
  applied with DVE multiplies.  Numerators copy to SBUF on GpSimd.
- PSUM->SBUF projection copies (+bias add) run on GpSimd; out staging
  round-robins Vector/Scalar/GpSimd.
"""

import sys

sys.path.insert(0, "/opt/trn_rl_repo")

import numpy as np

B, T, H = 2, 2048, 1024
NH, DK = 16, 64
HPC = 4            # heads per core
NCORES = 8
TK = 1088          # compacted+padded key length (8.5 tiles of 128)
KH = H // 128      # 8 contraction tiles
NQT = T // 128     # 16 token tiles

_CACHE = {}


def _build(tk=TK):
    import concourse.bacc as bacc
    import concourse.mybir as mybir
    import concourse.tile as tile

    f32 = mybir.dt.float32
    f32r = mybir.dt.float32r
    bf16 = mybir.dt.bfloat16
    AF = mybir.ActivationFunctionType
    ALU = mybir.AluOpType

    nkt = (tk + 127) // 128    # key tiles (last one may be half)

    nc = bacc.Bacc("TRN2", target_bir_lowering=False, debug=False)

    xt_d = nc.dram_tensor("xt", [128, KH * T], bf16, kind="ExternalInput")
    xkv_d = nc.dram_tensor("xkv", [128, KH * tk], bf16, kind="ExternalInput")
    wqk_d = nc.dram_tensor("wqk", [128, KH * 512], bf16, kind="ExternalInput")
    wv_d = nc.dram_tensor("wv", [128, KH * 256], bf16, kind="ExternalInput")
    wout_d = nc.dram_tensor("wout", [128, 2 * H], bf16, kind="ExternalInput")
    qkb_d = nc.dram_tensor("qkb", [128, 4], f32, kind="ExternalInput")
    vb_d = nc.dram_tensor("vb", [128, 256], f32, kind="ExternalInput")
    maskb_d = nc.dram_tensor("maskb", [128, nkt], f32, kind="ExternalInput")
    sel_d = nc.dram_tensor("sel", [97, 4 * 64], bf16, kind="ExternalInput")
    out_d = nc.dram_tensor("out_partial", [T, H], bf16, kind="ExternalOutput")

    with tile.TileContext(nc) as tc:
        with (
            tc.tile_pool(name="persist", bufs=1) as pp,
            tc.tile_pool(name="expp", bufs=3) as ep,
            tc.tile_pool(name="ostage", bufs=3) as osp,
            tc.tile_pool(name="psum", bufs=1, space="PSUM") as psp,
        ):
            # ---- persistent inputs ----
            # priority order: K/V-projection inputs first (xkv, wqk, wv),
            # then Q input xt, then out-projection weights.  xt (4MB)
            # otherwise steals HBM bandwidth from the startup-critical xkv.
            # per-contraction-chunk DMAs: projection matmul kt waits only
            # on chunk kt (Tile region tracking), so the PE starts as soon
            # as the first chunks land instead of after whole tensors.
            wqk = pp.tile([128, KH * 512], bf16, tag="wqk", name="wqk")
            wv = pp.tile([128, KH * 256], bf16, tag="wv", name="wv")
            xkv = pp.tile([128, KH * tk], bf16, tag="xkv", name="xkv")
            for kt in range(KH):
                nc.sync.dma_start(
                    out=xkv[:, kt * tk : (kt + 1) * tk],
                    in_=xkv_d[:, kt * tk : (kt + 1) * tk],
                )
                nc.gpsimd.dma_start(
                    out=wqk[:, kt * 512 : (kt + 1) * 512],
                    in_=wqk_d[:, kt * 512 : (kt + 1) * 512],
                )
                nc.gpsimd.dma_start(
                    out=wv[:, kt * 256 : (kt + 1) * 256],
                    in_=wv_d[:, kt * 256 : (kt + 1) * 256],
                )
            qkb = pp.tile([128, 4], f32, tag="qkb", name="qkb")
            nc.gpsimd.dma_start(out=qkb, in_=qkb_d[:, :])
            vb = pp.tile([128, 256], f32, tag="vb", name="vb")
            nc.gpsimd.dma_start(out=vb, in_=vb_d[:, :])
            maskb = pp.tile([128, nkt], f32, tag="maskb", name="maskb")
            nc.gpsimd.dma_start(out=maskb, in_=maskb_d[:, :])
            xt = pp.tile([128, KH * T], bf16, tag="xt", name="xt")
            for kt in range(KH):
                eng = nc.sync if kt % 2 == 0 else nc.gpsimd
                eng.dma_start(
                    out=xt[:, kt * T : (kt + 1) * T],
                    in_=xt_d[:, kt * T : (kt + 1) * T],
                )
            wout = pp.tile([128, 2 * H], bf16, tag="wout", name="wout")
            nc.sync.dma_start(out=wout, in_=wout_d[:, :])

            # selector-broadcast stationaries: sel[:, j*64:(j+1)*64] is
            # e_{32j} ⊗ ones64, so sel_j.T @ drec-block broadcasts the
            # drec row at partition 32j.  Engine SBUF writes may only start
            # at partitions {0,32,64,96}, so denominator rows are scattered
            # over those partitions x 4 column blocks.
            sel = pp.tile([97, 4 * 64], bf16, tag="sel", name="sel")
            nc.gpsimd.dma_start(out=sel, in_=sel_d[:, :])

            # ---- persistent intermediates ----
            qT = [pp.tile([128, T], bf16, tag=f"qT{p}", name=f"qT{p}") for p in range(2)]
            kT = [pp.tile([128, tk], bf16, tag=f"kT{p}", name=f"kT{p}") for p in range(2)]
            vnat = pp.tile([128, nkt * 260], bf16, tag="vnat", name="vnat")
            nc.gpsimd.memset(vnat, 1.0)  # ones columns; data cols overwritten
            attn = [
                pp.tile([128, T], bf16, tag=f"attn{p}", name=f"attn{p}")
                for p in range(2)
            ]
            # one [65,512] slice per (pair,nb,lh): rows 0-63 numerators,
            # row 64 the softmax denominator (copied out of PSUM in a single
            # DVE op so the acc bank frees fast).
            accS = pp.tile([65, 16 * 512], bf16, tag="accS", name="accS")
            dall = pp.tile([97, 2048], f32, tag="dall", name="dall")
            dscr = pp.tile([97, 2048], f32, tag="dscr", name="dscr")
            drec = pp.tile([97, 2048], bf16, tag="drec", name="drec")
            # reciprocal/sel-MM read all 97 partitions; the rows between
            # the 4 used ones must hold finite values, not NaN patterns.
            nc.vector.memset(dall, 1.0)

            # wqk column layout per kt chunk: [q01 | k01 | q23 | k23] * 128
            QM = {0: 0, 1: 2}   # pair -> wqk mtile index for q
            KM = {0: 1, 1: 3}   # pair -> wqk mtile index for k

            def proj_q_group(pair, nb):
                ps = psp.tile([128, 512], f32, tag="ps", bufs=2, name="ps")
                m = QM[pair]
                for kt in range(KH):
                    nc.tensor.matmul(
                        ps,
                        wqk[:, kt * 512 + m * 128 : kt * 512 + (m + 1) * 128],
                        xt[:, kt * T + nb * 512 : kt * T + nb * 512 + 512],
                        start=(kt == 0),
                        stop=(kt == KH - 1),
                    )
                nc.vector.tensor_scalar(
                    out=qT[pair][:, nb * 512 : (nb + 1) * 512],
                    in0=ps,
                    scalar1=qkb[:, m : m + 1],
                    scalar2=None,
                    op0=ALU.add,
                )

            KBLK = [(o, min(512, tk - o)) for o in range(0, tk, 512)]

            def proj_k_group(pair, blk):
                off, size = KBLK[blk]
                ps = psp.tile([128, 512], f32, tag="ps", bufs=2, name="ps")
                m = KM[pair]
                for kt in range(KH):
                    nc.tensor.matmul(
                        ps[:, 0:size],
                        wqk[:, kt * 512 + m * 128 : kt * 512 + (m + 1) * 128],
                        xkv[:, kt * tk + off : kt * tk + off + size],
                        start=(kt == 0),
                        stop=(kt == KH - 1),
                    )
                nc.vector.tensor_scalar(
                    out=kT[pair][:, off : off + size],
                    in0=ps[:, 0:size],
                    scalar1=qkb[:, m : m + 1],
                    scalar2=None,
                    op0=ALU.add,
                )

            def proj_v_group(tt):
                pt = min(128, tk - tt * 128)
                ps = psp.tile([128, 512], f32, tag="ps", bufs=2, name="ps")
                for kt in range(KH):
                    nc.tensor.matmul(
                        ps[0:pt, 0:256],
                        xkv[:, kt * tk + tt * 128 : kt * tk + tt * 128 + pt],
                        wv[:, kt * 256 : (kt + 1) * 256],
                        start=(kt == 0),
                        stop=(kt == KH - 1),
                    )
                vv = vnat[0:pt, tt * 260 : (tt + 1) * 260].rearrange(
                    "p (h c) -> p h c", c=65
                )
                nc.vector.tensor_tensor(
                    out=vv[:, :, 0:64],
                    in0=ps[0:pt, 0:256].rearrange("p (h c) -> p h c", c=64),
                    in1=vb[0:pt, :].rearrange("p (h c) -> p h c", c=64),
                    op=ALU.add,
                )

            def attention_pair(hp, interleave, mid_cb=None, pops=(2, 2, 2, 2)):
                # interleave: list of zero-arg emitters run between nb blocks
                il = list(interleave)
                for nb in range(4):
                    if nb == 2 and mid_cb is not None:
                        mid_cb()
                    for _ in range(pops[nb]):
                        if il:
                            il.pop(0)()
                    accs = [
                        psp.tile([65, 512], f32, tag="acc", bufs=2, name="acc")
                        for _ in range(2)
                    ]
                    pend = None  # deferred AV emitter (software pipeline by 1)
                    for kt in range(nkt):
                        pk = min(128, tk - kt * 128)
                        ss = psp.tile([128, 1024], f32, tag="ss", bufs=2, name="ss")
                        for lh in range(2):
                            r0 = lh * 64
                            nc.tensor.matmul(
                                ss[0:pk, lh * 512 : (lh + 1) * 512],
                                kT[hp][r0 : r0 + 64, kt * 128 : kt * 128 + pk],
                                qT[hp][r0 : r0 + 64, nb * 512 : nb * 512 + 512],
                                start=True,
                                stop=True,
                            )
                        ex = ep.tile([128, 1024], bf16, tag="ex", name="ex")
                        nc.scalar.activation(
                            ex[0:pk, :], ss[0:pk, :], AF.Exp,
                            bias=maskb[0:pk, kt : kt + 1],
                            scale=0.125,
                        )
                        if pend is not None:
                            pend()
                        def av(kt=kt, ex=ex, pk=pk):
                            for lh in range(2):
                                h = hp * 2 + lh
                                nc.tensor.matmul(
                                    accs[lh],
                                    vnat[0:pk, kt * 260 + h * 65 : kt * 260 + (h + 1) * 65],
                                    ex[0:pk, lh * 512 : (lh + 1) * 512],
                                    start=(kt == 0),
                                    stop=(kt == nkt - 1),
                                )
                        pend = av
                    pend()
                    # single [65,512] PSUM->SBUF copy per acc (fast bank
                    # release); the last block goes to ACT (idle after the
                    # final exp) so tail mults aren't queued behind it on
                    # DVE.  The denominator row then moves to its dall slot
                    # on the otherwise idle GpSimd (SBUF->SBUF).
                    last = nb == 3
                    for lh in range(2):
                        j2 = nb * 2 + lh
                        idx = hp * 8 + j2
                        p0 = 32 * (j2 % 4)
                        blk = hp * 2 + j2 // 4
                        dst = accS[:, idx * 512 : (idx + 1) * 512]
                        if last:
                            nc.scalar.copy(dst, accs[lh])
                        else:
                            nc.vector.tensor_copy(dst, accs[lh])
                        nc.vector.tensor_copy(
                            dall[p0 : p0 + 1, blk * 512 : blk * 512 + 512],
                            accS[64:65, idx * 512 : (idx + 1) * 512],
                        )
                while il:
                    il.pop(0)()

            def recip_cols(c0, c1):
                # ~18-bit approx is plenty for bf16 denominators and ~5x
                # cheaper than the exact InstReciprocal on the DVE.
                nc.vector.reciprocal_approx_fast(
                    out=dscr[:, c0:c1], in_=dall[:, c0:c1]
                )
                nc.vector.tensor_copy(drec[:, c0:c1], dscr[:, c0:c1])

            def normalize_nb(hp, nb):
                j = hp * 4 + nb
                for lh in range(2):
                    j2 = nb * 2 + lh
                    p0s = j2 % 4          # sel block (partition 32*p0s)
                    blk = hp * 2 + j2 // 4
                    r0 = lh * 64
                    pb = psp.tile([128, 512], f32, tag="ps", bufs=2, name="pb")
                    nc.tensor.matmul(
                        pb[0:64, :],
                        sel[:, p0s * 64 : (p0s + 1) * 64],
                        drec[:, blk * 512 : blk * 512 + 512],
                        start=True, stop=True,
                    )
                    idx = hp * 8 + j2
                    nc.vector.tensor_tensor(
                        out=attn[hp][r0 : r0 + 64, nb * 512 : nb * 512 + 512],
                        in0=accS[0:64, idx * 512 : (idx + 1) * 512],
                        in1=pb[0:64, :],
                        op=ALU.mult,
                    )

            def outproj(mts):
                for mt in mts:
                    po = psp.tile([128, 1024], f32, tag="ss", bufs=2, name="po")
                    for ob in range(2):
                        for p in range(2):
                            nc.tensor.matmul(
                                po[:, ob * 512 : ob * 512 + 512],
                                attn[p][:, mt * 128 : (mt + 1) * 128],
                                wout[:, p * H + ob * 512 : p * H + ob * 512 + 512],
                                start=(p == 0),
                                stop=(p == 1),
                            )
                    ot = osp.tile([128, 1024], bf16, tag="ot", name="ot")
                    if mt % 2 == 0:
                        nc.vector.tensor_copy(ot, po)
                    else:
                        nc.scalar.copy(ot, po)
                    nc.sync.dma_start(
                        out=out_d[mt * 128 : (mt + 1) * 128, :], in_=ot
                    )

            # ---- schedule ----
            # Pre-attention: only what pair-0 attention needs (k01, V, q01);
            # k23/q23 projection and pair-0 normalization interleave with
            # the ACT-bound attention loops so the PE never idles long
            # enough for HAM to re-throttle.
            nkb = len(KBLK)
            for blk in range(nkb):
                proj_k_group(0, blk)
            for tt in range(nkt):
                proj_v_group(tt)
            proj_q_group(0, 0)

            fills0 = [
                (lambda nb=nb: proj_q_group(0, nb)) for nb in (1, 2, 3)
            ] + [
                (lambda blk=blk: proj_k_group(1, blk)) for blk in range(nkb)
            ] + [
                (lambda nb=nb: proj_q_group(1, nb)) for nb in range(3)
            ]
            attention_pair(0, fills0, pops=(1, 1, 2, 2))
            # all of q23's projection copies must precede the reciprocal in
            # the DVE FIFO, or pair-1 attention stalls on its q tiles.
            proj_q_group(1, 3)
            # pair-0 denominators complete: reciprocal runs on DVE while the
            # PE streams pair-1 attention; pair-0 normalize MMs fill its gaps
            # (not at nb 0 -- their pb MMs would stall on it).
            recip_cols(0, 1024)
            fills1 = [
                (lambda nb=nb: normalize_nb(0, nb)) for nb in range(4)
            ]
            # pair-1 block-2 denominators (nb 0/1) ready after nb==1; their
            # reciprocal overlaps nb 2/3.
            attention_pair(
                1, fills1,
                mid_cb=lambda: recip_cols(1024, 1536),
                pops=(0, 1, 2, 1),
            )
            normalize_nb(1, 0)
            normalize_nb(1, 1)
            recip_cols(1536, 2048)
            # outproj token tiles 0-7 read only attn cols 0:1024 (nb 0/1),
            # keeping the PE busy while the block-3 reciprocal runs.
            outproj(range(0, 8))
            normalize_nb(1, 2)
            normalize_nb(1, 3)
            outproj(range(8, NQT))

    nc.compile()
    return nc


def _get_nc(tk=TK):
    key = f"nc{tk}"
    if key not in _CACHE:
        _CACHE[key] = _build(tk)
    return _CACHE[key]


def _prep_in_maps(x, mask, W_qkv, b_qkv, W_out, tk=TK):
    import ml_dtypes

    bf16 = ml_dtypes.bfloat16

    in_maps = []
    # per-batch compacted kv token sets
    kv_idx = []
    for b in range(B):
        idx = np.nonzero(mask[b, 0, 0, :] != 0)[0]
        assert len(idx) <= tk
        kv_idx.append(idx)

    for c in range(NCORES):
        b = c // 4
        h0 = (c % 4) * HPC
        xb = np.asarray(x[b], dtype=np.float32)

        # xt: [128, KH*T], chunk kt cols = x[b][:, kt*128:+128].T
        xT = np.ascontiguousarray(xb.T).astype(bf16)          # [H, T]
        xt_t = xT.reshape(KH, 128, T).transpose(1, 0, 2).reshape(128, KH * T)

        # compacted kv tokens, padded to tk
        idx = kv_idx[b]
        xkvb = np.zeros((tk, H), dtype=np.float32)
        xkvb[: len(idx)] = xb[idx]
        xkvT = np.ascontiguousarray(xkvb.T).astype(bf16)      # [H, tk]
        xkv_t = xkvT.reshape(KH, 128, tk).transpose(1, 0, 2).reshape(128, KH * tk)

        # wqk: per kt chunk [q01|k01|q23|k23]*128
        cols = []
        for pair in range(2):
            cols.append(np.arange(0 * H + (h0 + 2 * pair) * DK,
                                  0 * H + (h0 + 2 * pair + 2) * DK))
        qcols = [cols[0], cols[1]]
        kcols = [c_ + H for c_ in qcols]
        mcols = np.concatenate([qcols[0], kcols[0], qcols[1], kcols[1]])
        wqk_full = np.asarray(W_qkv, dtype=np.float32)[:, mcols]   # [H, 512]
        wqk_t = (
            wqk_full.astype(bf16).reshape(KH, 128, 512)
            .transpose(1, 0, 2).reshape(128, KH * 512)
        )

        vcols = np.arange(2 * H + h0 * DK, 2 * H + (h0 + HPC) * DK)
        wv_full = np.asarray(W_qkv, dtype=np.float32)[:, vcols]    # [H, 256]
        wv_t = (
            wv_full.astype(bf16).reshape(KH, 128, 256)
            .transpose(1, 0, 2).reshape(128, KH * 256)
        )

        wout_sl = np.asarray(W_out, dtype=np.float32)[
            h0 * DK : (h0 + HPC) * DK, :
        ]  # [256, H]
        wout_t = np.concatenate(
            [wout_sl[0:128, :], wout_sl[128:256, :]], axis=1
        ).astype(bf16)  # [128, 2H]

        bq = np.asarray(b_qkv, dtype=np.float32)
        qkb_t = np.stack(
            [bq[mcols[m * 128 : (m + 1) * 128]] for m in range(4)], axis=1
        )  # [128, 4]
        vb_t = np.broadcast_to(bq[vcols], (128, 256)).copy()  # [128, 256]

        nu = len(idx)
        nkt = (tk + 127) // 128
        mb = np.zeros((128, nkt), dtype=np.float32)
        flat = np.arange(nkt * 128).reshape(nkt, 128).T  # [128, nkt]
        mb[flat >= nu] = -1e9

        sel = np.zeros((97, 4 * 64), dtype=ml_dtypes.bfloat16)
        for j in range(4):
            sel[32 * j, j * 64 : (j + 1) * 64] = 1.0

        in_maps.append(
            {
                "xt": np.ascontiguousarray(xt_t),
                "xkv": np.ascontiguousarray(xkv_t),
                "wqk": np.ascontiguousarray(wqk_t),
                "wv": np.ascontiguousarray(wv_t),
                "wout": np.ascontiguousarray(wout_t),
                "qkb": np.ascontiguousarray(qkb_t),
                "vb": np.ascontiguousarray(vb_t),
                "maskb": np.ascontiguousarray(mb),
                "sel": sel,
            }
        )
    return in_maps


def _combine(partials, b_out):
    out = np.empty((B, T, H), dtype=np.float32)
    for b in range(B):
        acc = partials[4 * b].astype(np.float32)
        for i in range(1, 4):
            acc = acc + partials[4 * b + i]
        out[b] = acc + np.asarray(b_out, dtype=np.float32)[None, :]
    return out


def kernel(x, mask, W_qkv, b_qkv, W_out, b_out):
    x = np.asarray(x, dtype=np.float32)
    mask = np.asarray(mask)
    W_qkv = np.asarray(W_qkv, dtype=np.float32)
    b_qkv = np.asarray(b_qkv, dtype=np.float32)
    W_out = np.asarray(W_out, dtype=np.float32)
    b_out = np.asarray(b_out, dtype=np.float32)

    # compaction capacity check (always true for the reference inputs);
    # fall back to an uncompacted build if a mask is unusually dense.
    counts = [int((mask[b, 0, 0, :] != 0).sum()) for b in range(B)]
    tk = TK if max(counts) <= TK else T

    nc = _get_nc(tk)
    in_maps = _prep_in_maps(x, mask, W_qkv, b_qkv, W_out, tk)

    from concourse.bass_utils import run_bass_kernel_spmd

    res = run_bass_kernel_spmd(nc, in_maps, list(range(NCORES)))
    partials = [res.results[c]["out_partial"] for c in range(NCORES)]
    return _combine(partials, b_out)


# revision 33
# speedup vs baseline: 1.0214x; 1.0214x over previous
"""Multi-head attention (B=2, T=2048, H=1024, 16 heads) on 8 trn2 cores.

Sharding: data-parallel over batch (2) x tensor-parallel over head groups
(4 heads/core).  Each core computes qkv projection for its 4 heads,
attention, and a partial out-projection; the host sums 4 partials per
batch and adds b_out.

Key optimizations over the f32r v1 kernel:
- The mask zeroes entire key tokens for every query, so the host compacts
  K/V inputs to the ~1024 unmasked tokens, padded to TK=1152 (9 tiles).
  Scores, exp and AV shrink by 7/16.  Pad slots get maskbias=-1e9.
- All matmul operands are bf16 (PE rate is the same as f32r but FWL
  halves weight loads, and SBUF/DMA traffic halves).  PSUM stays f32.
- x is transposed on the host; no on-device transposes at all.  V is
  projected directly in natural [token, feat] orientation.
- Scores for the two heads of a pair run as row-tiled K=64 matmuls
  (PE partitions 0-63 / 64-127) writing one [128,1024] PSUM tile; a
  single exp activation (mask bias per partition) covers both heads.
- V carries a ones column so the AV matmul accumulates the softmax
  denominator in row 64 of the PSUM acc; denominator rows are copied to
  SBUF (DVE), batched into one reciprocal per pair, PE-broadcast, and
  applied with DVE multiplies.  Numerators copy to SBUF on GpSimd.
- PSUM->SBUF projection copies (+bias add) run on GpSimd; out staging
  round-robins Vector/Scalar/GpSimd.
"""

import sys

sys.path.insert(0, "/opt/trn_rl_repo")

import numpy as np

B, T, H = 2, 2048, 1024
NH, DK = 16, 64
HPC = 4            # heads per core
NCORES = 8
TK = 1088          # compacted+padded key length (8.5 tiles of 128)
KH = H // 128      # 8 contraction tiles
NQT = T // 128     # 16 token tiles

_CACHE = {}


def _build(tk=TK):
    import concourse.bacc as bacc
    import concourse.mybir as mybir
    import concourse.tile as tile

    f32 = mybir.dt.float32
    f32r = mybir.dt.float32r
    bf16 = mybir.dt.bfloat16
    AF = mybir.ActivationFunctionType
    ALU = mybir.AluOpType

    nkt = (tk + 127) // 128    # key tiles (last one may be half)

    nc = bacc.Bacc("TRN2", target_bir_lowering=False, debug=False)

    xt_d = nc.dram_tensor("xt", [128, KH * T], bf16, kind="ExternalInput")
    xkv_d = nc.dram_tensor("xkv", [128, KH * tk], bf16, kind="ExternalInput")
    wqk_d = nc.dram_tensor("wqk", [128, KH * 512], bf16, kind="ExternalInput")
    wv_d = nc.dram_tensor("wv", [128, KH * 256], bf16, kind="ExternalInput")
    wout_d = nc.dram_tensor("wout", [128, 2 * H], bf16, kind="ExternalInput")
    qkb_d = nc.dram_tensor("qkb", [128, 4], f32, kind="ExternalInput")
    vb_d = nc.dram_tensor("vb", [128, 256], f32, kind="ExternalInput")
    maskb_d = nc.dram_tensor("maskb", [128, nkt], f32, kind="ExternalInput")
    sel_d = nc.dram_tensor("sel", [97, 4 * 64], bf16, kind="ExternalInput")
    out_d = nc.dram_tensor("out_partial", [T, H], bf16, kind="ExternalOutput")

    with tile.TileContext(nc) as tc:
        with (
            tc.tile_pool(name="persist", bufs=1) as pp,
            tc.tile_pool(name="expp", bufs=3) as ep,
            tc.tile_pool(name="ostage", bufs=3) as osp,
            tc.tile_pool(name="psum", bufs=1, space="PSUM") as psp,
        ):
            # ---- persistent inputs ----
            # priority order: K/V-projection inputs first (xkv, wqk, wv),
            # then Q input xt, then out-projection weights.  xt (4MB)
            # otherwise steals HBM bandwidth from the startup-critical xkv.
            # per-contraction-chunk DMAs: projection matmul kt waits only
            # on chunk kt (Tile region tracking), so the PE starts as soon
            # as the first chunks land instead of after whole tensors.
            wqk = pp.tile([128, KH * 512], bf16, tag="wqk", name="wqk")
            wv = pp.tile([128, KH * 256], bf16, tag="wv", name="wv")
            xkv = pp.tile([128, KH * tk], bf16, tag="xkv", name="xkv")
            for kt in range(KH):
                nc.sync.dma_start(
                    out=xkv[:, kt * tk : (kt + 1) * tk],
                    in_=xkv_d[:, kt * tk : (kt + 1) * tk],
                )
                nc.scalar.dma_start(
                    out=wqk[:, kt * 512 : (kt + 1) * 512],
                    in_=wqk_d[:, kt * 512 : (kt + 1) * 512],
                )
                nc.scalar.dma_start(
                    out=wv[:, kt * 256 : (kt + 1) * 256],
                    in_=wv_d[:, kt * 256 : (kt + 1) * 256],
                )
            qkb = pp.tile([128, 4], f32, tag="qkb", name="qkb")
            nc.scalar.dma_start(out=qkb, in_=qkb_d[:, :])
            vb = pp.tile([128, 256], f32, tag="vb", name="vb")
            nc.scalar.dma_start(out=vb, in_=vb_d[:, :])
            maskb = pp.tile([128, nkt], f32, tag="maskb", name="maskb")
            nc.scalar.dma_start(out=maskb, in_=maskb_d[:, :])
            xt = pp.tile([128, KH * T], bf16, tag="xt", name="xt")
            for kt in range(KH):
                eng = nc.sync if kt % 2 == 0 else nc.scalar
                eng.dma_start(
                    out=xt[:, kt * T : (kt + 1) * T],
                    in_=xt_d[:, kt * T : (kt + 1) * T],
                )
            wout = pp.tile([128, 2 * H], bf16, tag="wout", name="wout")
            nc.sync.dma_start(out=wout, in_=wout_d[:, :])

            # selector-broadcast stationaries: sel[:, j*64:(j+1)*64] is
            # e_{32j} ⊗ ones64, so sel_j.T @ drec-block broadcasts the
            # drec row at partition 32j.  Engine SBUF writes may only start
            # at partitions {0,32,64,96}, so denominator rows are scattered
            # over those partitions x 4 column blocks.
            sel = pp.tile([97, 4 * 64], bf16, tag="sel", name="sel")
            nc.scalar.dma_start(out=sel, in_=sel_d[:, :])

            # ---- persistent intermediates ----
            qT = [pp.tile([128, T], bf16, tag=f"qT{p}", name=f"qT{p}") for p in range(2)]
            kT = [pp.tile([128, tk], bf16, tag=f"kT{p}", name=f"kT{p}") for p in range(2)]
            vnat = pp.tile([128, nkt * 260], bf16, tag="vnat", name="vnat")
            nc.gpsimd.memset(vnat, 1.0)  # ones columns; data cols overwritten
            attn = [
                pp.tile([128, T], bf16, tag=f"attn{p}", name=f"attn{p}")
                for p in range(2)
            ]
            # one [65,512] slice per (pair,nb,lh): rows 0-63 numerators,
            # row 64 the softmax denominator (copied out of PSUM in a single
            # DVE op so the acc bank frees fast).
            accS = pp.tile([65, 16 * 512], bf16, tag="accS", name="accS")
            dall = pp.tile([97, 2048], f32, tag="dall", name="dall")
            dscr = pp.tile([97, 2048], f32, tag="dscr", name="dscr")
            drec = pp.tile([97, 2048], bf16, tag="drec", name="drec")
            # reciprocal/sel-MM read all 97 partitions; the rows between
            # the 4 used ones must hold finite values, not NaN patterns.
            nc.vector.memset(dall, 1.0)

            # wqk column layout per kt chunk: [q01 | k01 | q23 | k23] * 128
            QM = {0: 0, 1: 2}   # pair -> wqk mtile index for q
            KM = {0: 1, 1: 3}   # pair -> wqk mtile index for k

            def proj_q_group(pair, nb):
                ps = psp.tile([128, 512], f32, tag="ps", bufs=2, name="ps")
                m = QM[pair]
                for kt in range(KH):
                    nc.tensor.matmul(
                        ps,
                        wqk[:, kt * 512 + m * 128 : kt * 512 + (m + 1) * 128],
                        xt[:, kt * T + nb * 512 : kt * T + nb * 512 + 512],
                        start=(kt == 0),
                        stop=(kt == KH - 1),
                    )
                nc.vector.tensor_scalar(
                    out=qT[pair][:, nb * 512 : (nb + 1) * 512],
                    in0=ps,
                    scalar1=qkb[:, m : m + 1],
                    scalar2=None,
                    op0=ALU.add,
                )

            KBLK = [(o, min(512, tk - o)) for o in range(0, tk, 512)]

            def proj_k_group(pair, blk):
                off, size = KBLK[blk]
                ps = psp.tile([128, 512], f32, tag="ps", bufs=2, name="ps")
                m = KM[pair]
                for kt in range(KH):
                    nc.tensor.matmul(
                        ps[:, 0:size],
                        wqk[:, kt * 512 + m * 128 : kt * 512 + (m + 1) * 128],
                        xkv[:, kt * tk + off : kt * tk + off + size],
                        start=(kt == 0),
                        stop=(kt == KH - 1),
                    )
                nc.vector.tensor_scalar(
                    out=kT[pair][:, off : off + size],
                    in0=ps[:, 0:size],
                    scalar1=qkb[:, m : m + 1],
                    scalar2=None,
                    op0=ALU.add,
                )

            def proj_v_group(tt):
                pt = min(128, tk - tt * 128)
                ps = psp.tile([128, 512], f32, tag="ps", bufs=2, name="ps")
                for kt in range(KH):
                    nc.tensor.matmul(
                        ps[0:pt, 0:256],
                        xkv[:, kt * tk + tt * 128 : kt * tk + tt * 128 + pt],
                        wv[:, kt * 256 : (kt + 1) * 256],
                        start=(kt == 0),
                        stop=(kt == KH - 1),
                    )
                vv = vnat[0:pt, tt * 260 : (tt + 1) * 260].rearrange(
                    "p (h c) -> p h c", c=65
                )
                nc.vector.tensor_tensor(
                    out=vv[:, :, 0:64],
                    in0=ps[0:pt, 0:256].rearrange("p (h c) -> p h c", c=64),
                    in1=vb[0:pt, :].rearrange("p (h c) -> p h c", c=64),
                    op=ALU.add,
                )

            def attention_pair(hp, interleave, mid_cb=None, pops=(2, 2, 2, 2)):
                # interleave: list of zero-arg emitters run between nb blocks
                il = list(interleave)
                for nb in range(4):
                    if nb == 2 and mid_cb is not None:
                        mid_cb()
                    for _ in range(pops[nb]):
                        if il:
                            il.pop(0)()
                    accs = [
                        psp.tile([65, 512], f32, tag="acc", bufs=2, name="acc")
                        for _ in range(2)
                    ]
                    pend = None  # deferred AV emitter (software pipeline by 1)
                    for kt in range(nkt):
                        pk = min(128, tk - kt * 128)
                        ss = psp.tile([128, 1024], f32, tag="ss", bufs=2, name="ss")
                        for lh in range(2):
                            r0 = lh * 64
                            nc.tensor.matmul(
                                ss[0:pk, lh * 512 : (lh + 1) * 512],
                                kT[hp][r0 : r0 + 64, kt * 128 : kt * 128 + pk],
                                qT[hp][r0 : r0 + 64, nb * 512 : nb * 512 + 512],
                                start=True,
                                stop=True,
                            )
                        ex = ep.tile([128, 1024], bf16, tag="ex", name="ex")
                        nc.scalar.activation(
                            ex[0:pk, :], ss[0:pk, :], AF.Exp,
                            bias=maskb[0:pk, kt : kt + 1],
                            scale=0.125,
                        )
                        if pend is not None:
                            pend()
                        def av(kt=kt, ex=ex, pk=pk):
                            for lh in range(2):
                                h = hp * 2 + lh
                                nc.tensor.matmul(
                                    accs[lh],
                                    vnat[0:pk, kt * 260 + h * 65 : kt * 260 + (h + 1) * 65],
                                    ex[0:pk, lh * 512 : (lh + 1) * 512],
                                    start=(kt == 0),
                                    stop=(kt == nkt - 1),
                                )
                        pend = av
                    pend()
                    # single [65,512] PSUM->SBUF copy per acc (fast bank
                    # release); the last block goes to ACT (idle after the
                    # final exp) so tail mults aren't queued behind it on
                    # DVE.  The denominator row then moves to its dall slot
                    # on the otherwise idle GpSimd (SBUF->SBUF).
                    last = nb == 3
                    for lh in range(2):
                        j2 = nb * 2 + lh
                        idx = hp * 8 + j2
                        p0 = 32 * (j2 % 4)
                        blk = hp * 2 + j2 // 4
                        dst = accS[:, idx * 512 : (idx + 1) * 512]
                        if last:
                            nc.scalar.copy(dst, accs[lh])
                        else:
                            nc.vector.tensor_copy(dst, accs[lh])
                        nc.vector.tensor_copy(
                            dall[p0 : p0 + 1, blk * 512 : blk * 512 + 512],
                            accS[64:65, idx * 512 : (idx + 1) * 512],
                        )
                while il:
                    il.pop(0)()

            def recip_cols(c0, c1):
                # ~18-bit approx is plenty for bf16 denominators and ~5x
                # cheaper than the exact InstReciprocal on the DVE.
                nc.vector.reciprocal_approx_fast(
                    out=dscr[:, c0:c1], in_=dall[:, c0:c1]
                )
                nc.vector.tensor_copy(drec[:, c0:c1], dscr[:, c0:c1])

            def normalize_nb(hp, nb):
                j = hp * 4 + nb
                for lh in range(2):
                    j2 = nb * 2 + lh
                    p0s = j2 % 4          # sel block (partition 32*p0s)
                    blk = hp * 2 + j2 // 4
                    r0 = lh * 64
                    pb = psp.tile([128, 512], f32, tag="ps", bufs=2, name="pb")
                    nc.tensor.matmul(
                        pb[0:64, :],
                        sel[:, p0s * 64 : (p0s + 1) * 64],
                        drec[:, blk * 512 : blk * 512 + 512],
                        start=True, stop=True,
                    )
                    idx = hp * 8 + j2
                    nc.vector.tensor_tensor(
                        out=attn[hp][r0 : r0 + 64, nb * 512 : nb * 512 + 512],
                        in0=accS[0:64, idx * 512 : (idx + 1) * 512],
                        in1=pb[0:64, :],
                        op=ALU.mult,
                    )

            def outproj(mts):
                for mt in mts:
                    po = psp.tile([128, 1024], f32, tag="ss", bufs=2, name="po")
                    for ob in range(2):
                        for p in range(2):
                            nc.tensor.matmul(
                                po[:, ob * 512 : ob * 512 + 512],
                                attn[p][:, mt * 128 : (mt + 1) * 128],
                                wout[:, p * H + ob * 512 : p * H + ob * 512 + 512],
                                start=(p == 0),
                                stop=(p == 1),
                            )
                    ot = osp.tile([128, 1024], bf16, tag="ot", name="ot")
                    if mt % 2 == 0:
                        nc.vector.tensor_copy(ot, po)
                    else:
                        nc.scalar.copy(ot, po)
                    nc.sync.dma_start(
                        out=out_d[mt * 128 : (mt + 1) * 128, :], in_=ot
                    )

            # ---- schedule ----
            # Pre-attention: only what pair-0 attention needs (k01, V, q01);
            # k23/q23 projection and pair-0 normalization interleave with
            # the ACT-bound attention loops so the PE never idles long
            # enough for HAM to re-throttle.
            nkb = len(KBLK)
            for blk in range(nkb):
                proj_k_group(0, blk)
            for tt in range(nkt):
                proj_v_group(tt)
            proj_q_group(0, 0)

            fills0 = [
                (lambda nb=nb: proj_q_group(0, nb)) for nb in (1, 2, 3)
            ] + [
                (lambda blk=blk: proj_k_group(1, blk)) for blk in range(nkb)
            ] + [
                (lambda nb=nb: proj_q_group(1, nb)) for nb in range(3)
            ]
            attention_pair(0, fills0, pops=(1, 1, 2, 2))
            # all of q23's projection copies must precede the reciprocal in
            # the DVE FIFO, or pair-1 attention stalls on its q tiles.
            proj_q_group(1, 3)
            # pair-0 denominators complete: reciprocal runs on DVE while the
            # PE streams pair-1 attention; pair-0 normalize MMs fill its gaps
            # (not at nb 0 -- their pb MMs would stall on it).
            recip_cols(0, 1024)
            fills1 = [
                (lambda nb=nb: normalize_nb(0, nb)) for nb in range(4)
            ]
            # pair-1 block-2 denominators (nb 0/1) ready after nb==1; their
            # reciprocal overlaps nb 2/3.
            attention_pair(
                1, fills1,
                mid_cb=lambda: recip_cols(1024, 1536),
                pops=(0, 1, 2, 1),
            )
            normalize_nb(1, 0)
            normalize_nb(1, 1)
            recip_cols(1536, 2048)
            # outproj token tiles 0-7 read only attn cols 0:1024 (nb 0/1),
            # keeping the PE busy while the block-3 reciprocal runs.
            outproj(range(0, 8))
            normalize_nb(1, 2)
            normalize_nb(1, 3)
            outproj(range(8, NQT))

    nc.compile()
    return nc


def _get_nc(tk=TK):
    key = f"nc{tk}"
    if key not in _CACHE:
        _CACHE[key] = _build(tk)
    return _CACHE[key]


def _prep_in_maps(x, mask, W_qkv, b_qkv, W_out, tk=TK):
    import ml_dtypes

    bf16 = ml_dtypes.bfloat16

    in_maps = []
    # per-batch compacted kv token sets
    kv_idx = []
    for b in range(B):
        idx = np.nonzero(mask[b, 0, 0, :] != 0)[0]
        assert len(idx) <= tk
        kv_idx.append(idx)

    for c in range(NCORES):
        b = c // 4
        h0 = (c % 4) * HPC
        xb = np.asarray(x[b], dtype=np.float32)

        # xt: [128, KH*T], chunk kt cols = x[b][:, kt*128:+128].T
        xT = np.ascontiguousarray(xb.T).astype(bf16)          # [H, T]
        xt_t = xT.reshape(KH, 128, T).transpose(1, 0, 2).reshape(128, KH * T)

        # compacted kv tokens, padded to tk
        idx = kv_idx[b]
        xkvb = np.zeros((tk, H), dtype=np.float32)
        xkvb[: len(idx)] = xb[idx]
        xkvT = np.ascontiguousarray(xkvb.T).astype(bf16)      # [H, tk]
        xkv_t = xkvT.reshape(KH, 128, tk).transpose(1, 0, 2).reshape(128, KH * tk)

        # wqk: per kt chunk [q01|k01|q23|k23]*128
        cols = []
        for pair in range(2):
            cols.append(np.arange(0 * H + (h0 + 2 * pair) * DK,
                                  0 * H + (h0 + 2 * pair + 2) * DK))
        qcols = [cols[0], cols[1]]
        kcols = [c_ + H for c_ in qcols]
        mcols = np.concatenate([qcols[0], kcols[0], qcols[1], kcols[1]])
        wqk_full = np.asarray(W_qkv, dtype=np.float32)[:, mcols]   # [H, 512]
        wqk_t = (
            wqk_full.astype(bf16).reshape(KH, 128, 512)
            .transpose(1, 0, 2).reshape(128, KH * 512)
        )

        vcols = np.arange(2 * H + h0 * DK, 2 * H + (h0 + HPC) * DK)
        wv_full = np.asarray(W_qkv, dtype=np.float32)[:, vcols]    # [H, 256]
        wv_t = (
            wv_full.astype(bf16).reshape(KH, 128, 256)
            .transpose(1, 0, 2).reshape(128, KH * 256)
        )

        wout_sl = np.asarray(W_out, dtype=np.float32)[
            h0 * DK : (h0 + HPC) * DK, :
        ]  # [256, H]
        wout_t = np.concatenate(
            [wout_sl[0:128, :], wout_sl[128:256, :]], axis=1
        ).astype(bf16)  # [128, 2H]

        bq = np.asarray(b_qkv, dtype=np.float32)
        qkb_t = np.stack(
            [bq[mcols[m * 128 : (m + 1) * 128]] for m in range(4)], axis=1
        )  # [128, 4]
        vb_t = np.broadcast_to(bq[vcols], (128, 256)).copy()  # [128, 256]

        nu = len(idx)
        nkt = (tk + 127) // 128
        mb = np.zeros((128, nkt), dtype=np.float32)
        flat = np.arange(nkt * 128).reshape(nkt, 128).T  # [128, nkt]
        mb[flat >= nu] = -1e9

        sel = np.zeros((97, 4 * 64), dtype=ml_dtypes.bfloat16)
        for j in range(4):
            sel[32 * j, j * 64 : (j + 1) * 64] = 1.0

        in_maps.append(
            {
                "xt": np.ascontiguousarray(xt_t),
                "xkv": np.ascontiguousarray(xkv_t),
                "wqk": np.ascontiguousarray(wqk_t),
                "wv": np.ascontiguousarray(wv_t),
                "wout": np.ascontiguousarray(wout_t),
                "qkb": np.ascontiguousarray(qkb_t),
                "vb": np.ascontiguousarray(vb_t),
                "maskb": np.ascontiguousarray(mb),
                "sel": sel,
            }
        )
    return in_maps


def _combine(partials, b_out):
    out = np.empty((B, T, H), dtype=np.float32)
    for b in range(B):
        acc = partials[4 * b].astype(np.float32)
        for i in range(1, 4):
            acc = acc + partials[4 * b + i]
        out[b] = acc + np.asarray(b_out, dtype=np.float32)[None, :]
    return out


def kernel(x, mask, W_qkv, b_qkv, W_out, b_out):
    x = np.asarray(x, dtype=np.float32)
    mask = np.asarray(mask)
    W_qkv = np.asarray(W_qkv, dtype=np.float32)
    b_qkv = np.asarray(b_qkv, dtype=np.float32)
    W_out = np.asarray(W_out, dtype=np.float32)
    b_out = np.asarray(b_out, dtype=np.float32)

    # compaction capacity check (always true for the reference inputs);
    # fall back to an uncompacted build if a mask is unusually dense.
    counts = [int((mask[b, 0, 0, :] != 0).sum()) for b in range(B)]
    tk = TK if max(counts) <= TK else T

    nc = _get_nc(tk)
    in_maps = _prep_in_maps(x, mask, W_qkv, b_qkv, W_out, tk)

    from concourse.bass_utils import run_bass_kernel_spmd

    res = run_bass_kernel_spmd(nc, in_maps, list(range(NCORES)))
    partials = [res.results[c]["out_partial"] for c in range(NCORES)]
    return _combine(partials, b_out)
